# revision 41
# baseline (speedup 1.0000x reference)
"""Trainium2 Bass kernel for a dense transformer block (B=8, T=1024, C=1024, H=16).

Data-parallel over batch across the 8 NeuronCores (one batch element per core,
weights replicated, no collectives).

v2: qkT / V / attn-proj matmuls run in fp8e4 DoubleRow (2 k-chunks per PE pass,
~1.7x on those GEMMs); fc1/fc2 stay bf16 (fp8 there blows the 2e-2 rel-err
budget; measured sim rel_l2 0.0117 for this split vs 0.0294 with fp8 MLP).
Scales: weights x32 host-side, yT x16 via the softmax-denominator reciprocal
(vaug ones-column = 1/16), un-scaled on eviction (qkT,V: DVE mult 1/32 replaces
the copy; proj: /512 on eviction).

Per-core dataflow:

  stage W   ~40 warmup matmuls on a zeroed tile (HAM un-throttle), own psum
            pool closed before the main pools so nothing serializes on it.
  stage A   per token chunk: LN1 (DVE stats, ACT normalize -> fp8) -> PE
            transpose (fp8) -> bufT8.  x chunk DMAs front-loaded; wv/wp fp8
            weight DMAs stream underneath.
  stage B   qkT chunks for head-pairs 0-3: fp8 DR, lhsT=wqk 2-chunk groups,
            rhs=bufT8 -> psum -> DVE mult 1/32 -> qk8 (fp8, scale 1).
  stage C   software-pipelined attention, pv lagged TWO pairs behind scores
            so PE never waits on the ACT-bound exp:
              scores: both heads of a pair on 64-row PE tiles from qk8 (fp8
                      non-DR, full rate); exp on ACT; diag masked post-exp.
              PV:     [65,512] psum chains, col 64 = denominator/16.
              filler: V chunks (DR) during pairs 0-1, qkT pairs 4-7 half
                      chains (DR) during pairs 1-5, proj k0-3 partials (DR,
                      eviction ps/512 + add on DVE) during pairs 6-7.
              den:    batched reciprocal (bf16) -> DRAM-broadcast -> yT8 =
                      yT * (16/den) in fp8.
  stage D   proj k4-7: k4,5 DR chains 2 deep; k6, k7 fp8 non-DR lagged behind
            the den tail; evict via ACT(scale 1/512)->f32 + DVE add; LN2 ->
            bf16 -> transpose -> bufT16.
  stage E   fc1 + gelu -> aT bf16 (unchanged from v1).
  stage F   fc2 + residual -> out, 2 column passes, 8x 1-bank psums.
"""
import sys

sys.path.insert(0, "/opt/trn_rl_repo")

import numpy as np
import ml_dtypes

import concourse.bass as bass
import concourse.tile as tile
from concourse import mybir
from concourse.masks import make_identity
from concourse.vector_clock import ScopedClock

F32 = mybir.dt.float32
BF16 = mybir.dt.bfloat16
F8 = mybir.dt.float8e4
AF = mybir.ActivationFunctionType
DR = mybir.MatmulPerfMode.DoubleRow
MUL = mybir.AluOpType.mult

T, C, H, D = 1024, 1024, 16, 64
NT = T // 128   # 8 token chunks
NC_ = C // 128  # 8 feature chunks
EPS = 1e-5
WS = 32.0       # fp8 weight scale for w_qk, w_v (stored x32, free DVE unscale)
# w_proj and yT8 are stored UNSCALED (subnormal-range fp8 quantization is as
# accurate as scaled for this data; sim rel_l2 0.0118 either way) so the proj
# psum is exact and its eviction is a single plain tensor_add.

# ---------------------------------------------------------------------------
# Walrus in this container rejects >1 sem-wait per CTRL instruction; split the
# Tile tail-drain's waits across nop carriers.
_MAX_WAITS = 1


def _patched_drain_and_barrier(self, tick_clock, wait_clock):
    nc = self.nc
    carrier = nc.sync.nop(nofuse=True)
    wait_clock.add_sem_waits(carrier.ins, ScopedClock({None: tick_clock.global_clock}))
    si = carrier.ins.sync_info
    waits = list(si.on_wait) if si and si.on_wait else []
    if len(waits) > _MAX_WAITS:
        si.on_wait = waits[:_MAX_WAITS]
        for k in range(_MAX_WAITS, len(waits), _MAX_WAITS):
            extra = nc.sync.nop(nofuse=True)
            esi = extra.ins.sync_info
            if esi is None:
                extra.ins.sync_info = mybir.SyncInfo(
                    on_wait=waits[k:k + _MAX_WAITS], on_update=[]
                )
            else:
                esi.on_wait = waits[k:k + _MAX_WAITS]
    nc.sync.drain()
    nc.all_engine_barrier()
    popped = nc._tile_sem_poison_stack.pop()
    assert popped is self._sem_poison
    nc.clear_and_free_semaphores(list(self.sems.allocated().values()))
    nc.all_engine_barrier()


tile.TileContext._drain_and_barrier = _patched_drain_and_barrier


def _split_sync_waits(nc, max_waits=1):
    """Walrus here rejects >1 sem-wait per instruction; hoist extras onto
    preceding same-engine nops."""
    ctr = 0
    for f in nc.m.functions:
        for b in f.blocks:
            out = []
            for ins in b.instructions:
                si = ins.sync_info
                ws = list(si.on_wait) if si and si.on_wait else []
                if len(ws) > max_waits:
                    extra, keep = ws[:-max_waits], ws[-max_waits:]
                    for i in range(0, len(extra), max_waits):
                        nop = mybir.InstNoOp(
                            name=f"wsplit-{ctr}", engine=ins.engine,
                            sync_info=mybir.SyncInfo(
                                on_wait=extra[i:i + max_waits], on_update=[]))
                        ctr += 1
                        out.append(nop)
                    si.on_wait = keep
                out.append(ins)
            b.instructions = out


def build_nc(flags, split_waits=True):
    nc = bass.Bass()

    x_d = nc.dram_tensor("x", [T, C], F32, kind="ExternalInput")
    # host-prearranged: [m_chunk, p, ko, 128] so per-chunk DMAs are contiguous
    wqk_d = nc.dram_tensor("w_qk", [2 * NC_, 128, NC_, 128], F8,
                           kind="ExternalInput")
    wfc_d = nc.dram_tensor("w_fc", [4 * NC_, 128, NC_, 128], BF16,
                           kind="ExternalInput")
    wv_d = nc.dram_tensor("w_v", [C, C], F8, kind="ExternalInput")
    wp_d = nc.dram_tensor("w_proj", [C, C], F8, kind="ExternalInput")
    # host-prearranged: [half, k, p, 512] column halves for the 2-pass fc2
    wmlp_d = nc.dram_tensor("w_mlp", [2, 4 * NC_, 128, 512], BF16,
                            kind="ExternalInput")
    mask_d = nc.dram_tensor("mask_ut", [128, 128], BF16, kind="ExternalInput")
    opt = {}
    if flags["b_qk"]:
        opt["b_qk"] = nc.dram_tensor("b_qk", [128, 2 * NC_], F32, kind="ExternalInput")
    if flags["b_v"]:
        opt["b_v"] = nc.dram_tensor("b_v", [C], F32, kind="ExternalInput")
    if flags["b_proj"]:
        opt["b_proj"] = nc.dram_tensor("b_proj", [C], F32, kind="ExternalInput")
    if flags["b_fc"]:
        opt["b_fc"] = nc.dram_tensor("b_fc", [128, 4 * NC_], F32, kind="ExternalInput")
    if flags["b_mlp"]:
        opt["b_mlp"] = nc.dram_tensor("b_mlp", [C], F32, kind="ExternalInput")
    for nm in ("ln1_g", "ln1_b", "ln2_g", "ln2_b"):
        if flags[nm]:
            opt[nm] = nc.dram_tensor(nm, [C], F32, kind="ExternalInput")
    out_d = nc.dram_tensor("out", [T, C], F32, kind="ExternalOutput")

    with tile.TileContext(nc) as tc:
        _build_body(nc, tc, flags, x_d, wqk_d, wv_d, wp_d, wfc_d, wmlp_d,
                    mask_d, opt, out_d)
    if split_waits:
        _split_sync_waits(nc)
    return nc


def _build_body(nc, tc, flags, x_d, wqk_d, wv_d, wp_d, wfc_d, wmlp_d,
                mask_d, opt, out_d):
    from contextlib import ExitStack

    ctx = ExitStack()
    with ctx:
        const = ctx.enter_context(tc.tile_pool(name="const", bufs=1))
        big = ctx.enter_context(tc.tile_pool(name="big", bufs=1))
        scratch = ctx.enter_context(tc.tile_pool(name="scratch", bufs=2))
        hb_pool = ctx.enter_context(tc.tile_pool(name="hbp", bufs=4))
        small = ctx.enter_context(tc.tile_pool(name="small", bufs=8))
        dram = ctx.enter_context(tc.tile_pool(name="dram", bufs=1, space="DRAM"))

        # ---- stage W: warmup matmuls on own psum pool (closed right away so
        # nothing rotates behind the warm tile) --------------------------------
        zw = const.tile([128, 128], BF16, tag="zw")
        nc.vector.memset(zw[:], 0.0)
        with tc.tile_pool(name="ps_warm", bufs=1, space="PSUM") as ps_warm:
            warm = ps_warm.tile([128, 128], F32, tag="warm", name="warm")
            for _ in range(56):
                nc.tensor.matmul(warm[:], zw[:], zw[:], start=True, stop=True)

        # ---- constants -----------------------------------------------------
        ident = const.tile([128, 128], BF16, tag="ident")
        make_identity(nc, ident)
        mask_sb = const.tile([128, 128], BF16, tag="mask")
        nc.sync.dma_start(mask_sb[:], mask_d[:])
        eps_t = const.tile([128, 1], F32, tag="eps")
        nc.vector.memset(eps_t[:], EPS)

        def rep128(vec_dram):
            t = const.tile([128, C], F32, tag=f"rep_{vec_dram.tensor.name}")
            src = bass.AP(tensor=vec_dram.tensor, offset=0, ap=[[0, 128], [1, C]])
            nc.gpsimd.dma_start(out=t[:], in_=src)
            return t

        ln1_g_rep = rep128(opt["ln1_g"]) if flags["ln1_g"] else None
        ln1_b_rep = rep128(opt["ln1_b"]) if flags["ln1_b"] else None
        ln2_g_rep = rep128(opt["ln2_g"]) if flags["ln2_g"] else None
        ln2_b_rep = rep128(opt["ln2_b"]) if flags["ln2_b"] else None
        bv_rep = rep128(opt["b_v"]) if flags["b_v"] else None
        bproj_rep = rep128(opt["b_proj"]) if flags["b_proj"] else None
        bmlp_rep = rep128(opt["b_mlp"]) if flags["b_mlp"] else None
        bqk_sb = None
        if flags["b_qk"]:
            bqk_sb = const.tile([128, 2 * NC_], F32, tag="bqk")
            nc.sync.dma_start(bqk_sb[:], opt["b_qk"][:])
        bfc_sb = None
        if flags["b_fc"]:
            bfc_sb = const.tile([128, 4 * NC_], F32, tag="bfc")
            nc.sync.dma_start(bfc_sb[:], opt["b_fc"][:])

        # ---- persistent tiles ---------------------------------------------
        x_sb = big.tile([128, NT, C], F32, tag="x")        # x, then r1 in place
        bufT8 = big.tile([128, NC_, T], F8, tag="bufT8")   # h1T fp8
        bufT16 = big.tile([128, NC_, T], BF16, tag="bufT16")  # h2T bf16
        yT = big.tile([128, NC_, T], BF16, tag="yT")       # raw attention out^T
        yT8 = big.tile([128, NC_, T], F8, tag="yT8")       # normalized, x16
        # all qkT chunks: [p, sub(q=0,k=1), pair, t], fp8 scale 1
        qk8 = big.tile([128, 2, NC_, T], F8, tag="qk8")

        def layernorm_chunk(src_slice, g_rep, b_rep, out_dt):
            stats = small.tile([128, 2, 6], F32, tag="bn_stats")
            xr = src_slice.rearrange("p (s f) -> p s f", f=512)
            for s in range(2):
                nc.vector.bn_stats(out=stats[:, s, :], in_=xr[:, s, :])
            mv = small.tile([128, 2], F32, tag="bn_mv")
            nc.vector.bn_aggr(out=mv[:], in_=stats[:])
            rstd = small.tile([128, 1], F32, tag="rstd")
            nc.scalar.activation(out=rstd[:], in_=mv[:, 1:2], func=AF.Sqrt,
                                 bias=eps_t[:], scale=1.0)
            nc.vector.reciprocal(out=rstd[:], in_=rstd[:])
            # nmr = -mu * rstd; normalize on ACT: h = x*rstd + nmr
            nmr = small.tile([128, 1], F32, tag="nmr")
            nc.vector.tensor_scalar(
                out=nmr[:], in0=mv[:, 0:1], scalar1=rstd[:], scalar2=-1.0,
                op0=MUL, op1=MUL)
            h_blk = hb_pool.tile([128, C], out_dt, tag="hblk")
            nc.scalar.activation(out=h_blk[:], in_=src_slice, func=AF.Identity,
                                 bias=nmr[:], scale=rstd[:])
            if g_rep is not None:
                nc.vector.tensor_mul(h_blk[:], h_blk[:], g_rep[:])
            if b_rep is not None:
                nc.vector.tensor_add(h_blk[:], h_blk[:], b_rep[:])
            return h_blk

        def transpose_into(ps_pool, dst, dst_ti, src_blk, idn, dt, use_act=False):
            for jc in range(NC_):
                pst = ps_pool.tile([128, 128], dt, tag="ps1b")
                nc.tensor.transpose(pst[:], src_blk[:, jc * 128:(jc + 1) * 128],
                                    idn[:])
                d = dst[:, jc, dst_ti * 128:(dst_ti + 1) * 128]
                if use_act and jc % 2 == 1:
                    # ACT has slack in stage D (only the LN2 normalize); DVE
                    # is the busy engine there.
                    nc.scalar.activation(out=d, in_=pst[:], func=AF.Copy)
                else:
                    eng = nc.vector if jc % 2 == 0 else nc.any
                    eng.tensor_copy(out=d, in_=pst[:])

        # Global PSUM pools for stages A-E: three 2-bank "main" slots (the
        # [128, <=1024] f32 accumulators: scores, V, proj chains, fc1) + two
        # 1-bank slots (transposes, PV halves, qkT half-chains, proj pass1)
        # = exactly 8 banks, no stage barriers.
        ps_ctx = ExitStack()
        ps_main = ps_ctx.enter_context(
            tc.tile_pool(name="ps_main", bufs=3, space="PSUM"))
        ps_sm = ps_ctx.enter_context(
            tc.tile_pool(name="ps_sm", bufs=2, space="PSUM"))

        wqks_ctx = ExitStack()
        wqks = wqks_ctx.enter_context(tc.tile_pool(name="wqks", bufs=2))

        def qkT_evict(ps, sub, c, m, off, n):
            if bqk_sb is not None:
                nc.scalar.activation(out=qk8[:, sub, c, off:off + n],
                                     in_=ps[:, off:off + n] if ps.shape[-1] > n
                                     else ps[:], func=AF.Identity,
                                     bias=bqk_sb[:, m:m + 1], scale=1.0 / WS)
            else:
                src = ps[:, off:off + n] if ps.shape[-1] > n else ps[:]
                nc.vector.tensor_scalar(
                    out=qk8[:, sub, c, off:off + n], in0=src,
                    scalar1=1.0 / WS, scalar2=None, op0=MUL)

        def qkT_chunk_full(sub, c):
            """Whole [128, T] chunk via a 2-bank psum, fp8 DoubleRow."""
            m = sub * NC_ + c
            wq = wqks.tile([128, NC_, 128], F8, tag="wq")
            nc.sync.dma_start(out=wq[:], in_=wqk_d[m])
            ps = ps_main.tile([128, T], F32, tag="psmain", name=f"psqk{m}")
            for g in range(NC_ // 2):
                lhsT = wq[:, 2 * g:2 * g + 2, :]
                for off in (0, 512):
                    nc.tensor.matmul(ps[:, off:off + 512], lhsT,
                                     bufT8[:, 2 * g:2 * g + 2, off:off + 512],
                                     start=(g == 0), stop=(g == NC_ // 2 - 1),
                                     perf_mode=DR)
            qkT_evict(ps, sub, c, m, 0, 512)
            qkT_evict(ps, sub, c, m, 512, 512)

        def qkT_chunk_half(sub, c, half):
            """One [128, 512] half-chain via a 1-bank psum (attention filler)."""
            m = sub * NC_ + c
            if half == 0:
                wq = wqks.tile([128, NC_, 128], F8, tag="wq", name=f"wq_{m}")
                nc.sync.dma_start(out=wq[:], in_=wqk_d[m])
                qkT_chunk_half.cur[m] = wq
            wq = qkT_chunk_half.cur[m]
            off = half * 512
            ps = ps_sm.tile([128, 512], F32, tag="ps1b", name=f"psqk{m}_{half}")
            for g in range(NC_ // 2):
                nc.tensor.matmul(ps[:], wq[:, 2 * g:2 * g + 2, :],
                                 bufT8[:, 2 * g:2 * g + 2, off:off + 512],
                                 start=(g == 0), stop=(g == NC_ // 2 - 1),
                                 perf_mode=DR)
            qkT_evict(ps, sub, c, m, off, 512)
        qkT_chunk_half.cur = {}

        # wp chunks (fp8): k=0..3 feed the in-attention proj partials, k=4..7
        # the post-attention pass.
        wplo_ctx = ExitStack()
        wplo = wplo_ctx.enter_context(tc.tile_pool(name="wplo", bufs=1))
        wp_lo = wplo.tile([128, 4, C], F8, tag="wplo")
        wps_ctx = ExitStack()
        wps = wps_ctx.enter_context(tc.tile_pool(name="wps", bufs=1))
        wp_hi = wps.tile([128, 4, C], F8, tag="wphi")

        def proj_evict_add(i, off, ps):
            nc.vector.tensor_add(x_sb[:, i, off:off + 512], ps[:],
                                 x_sb[:, i, off:off + 512])

        def proj_hi(i):
            """r1[i] partial: += y[:, k4..7] @ Wp rows, fp8 DR, 1-bank psums.
            Runs IN-attention: pairs 4-7 are processed first, so their den
            normalizations land mid-attention."""
            pss = []
            for off in (0, 512):
                pss.append(ps_sm.tile([128, 512], F32, tag="ps1b",
                                      name=f"pjh_{i}_{off}"))
            for g in (0, 1):
                lhsT = yT8[:, 4 + 2 * g:6 + 2 * g, i * 128:(i + 1) * 128]
                for oi, off in enumerate((0, 512)):
                    nc.tensor.matmul(pss[oi][:], lhsT,
                                     wp_hi[:, 2 * g:2 * g + 2, off:off + 512],
                                     start=(g == 0), stop=(g == 1),
                                     perf_mode=DR)
            for oi, off in enumerate((0, 512)):
                proj_evict_add(i, off, pss[oi])

        with tc.tile_pool(name="mid", bufs=1) as mid:
            vaug = mid.tile([128, NT, H, D + 1], BF16, tag="vaug")
            nc.vector.memset(vaug[:, :, :, D:D + 1], 1.0)
            wv_sb = mid.tile([128, NC_, C], F8, tag="wv")

            # ---- stage A: x DMAs, LN1 pipeline (2 ahead), transposes -------
            nc.sync.dma_start(out=x_sb[:, 0, :], in_=x_d[0:128, :])
            for ti in range(1, NT):
                nc.sync.dma_start(out=x_sb[:, ti, :],
                                  in_=x_d[ti * 128:(ti + 1) * 128, :])
            for k in range(NC_):
                nc.sync.dma_start(out=wv_sb[:, k, :],
                                  in_=wv_d[k * 128:(k + 1) * 128, :])
            for k in range(4):
                nc.sync.dma_start(out=wp_lo[:, k, :],
                                  in_=wp_d[k * 128:(k + 1) * 128, :])
                nc.sync.dma_start(out=wp_hi[:, k, :],
                                  in_=wp_d[(k + 4) * 128:(k + 5) * 128, :])
            # transpose runs in bf16 (walrus rejects fp8 transpose-mode psum
            # output); the DVE eviction copy converts bf16 psum -> fp8 bufT8.
            hb1 = {t: layernorm_chunk(x_sb[:, t, :], ln1_g_rep, ln1_b_rep,
                                      BF16) for t in range(3)}
            for ti in range(NT):
                if ti + 3 < NT:
                    hb1[ti + 3] = layernorm_chunk(
                        x_sb[:, ti + 3, :], ln1_g_rep, ln1_b_rep, BF16)
                transpose_into(ps_sm, bufT8, ti, hb1.pop(ti), ident, BF16)

            # dummy exp: pulls the ACT exp-table load into stage B (ACT idle)
            # instead of stalling the first scores eviction by ~2.5us
            dume = small.tile([128, 1], F32, tag="dume")
            nc.scalar.activation(out=dume[:], in_=eps_t[:], func=AF.Exp)

            # ---- stage B: qkT chunks for pairs 4-7 (dense DR; these pairs
            # are processed first in attention) -------------------------------
            for c in (4, 5, 6, 7):
                for sub in range(2):
                    qkT_chunk_full(sub, c)

            def v_chunk(ti):
                """V(ti) fp8 DR: lhsT=h1T chunk, rhs=Wv; evict *1/WS -> vaug."""
                ps = ps_main.tile([128, C], F32, tag="psmain", name=f"psv{ti}")
                for g in range(NC_ // 2):
                    lhsT = bufT8[:, 2 * g:2 * g + 2, ti * 128:(ti + 1) * 128]
                    for off in (0, 512):
                        nc.tensor.matmul(ps[:, off:off + 512], lhsT,
                                         wv_sb[:, 2 * g:2 * g + 2, off:off + 512],
                                         start=(g == 0),
                                         stop=(g == NC_ // 2 - 1),
                                         perf_mode=DR)
                ps3 = ps[:].rearrange("p (h d) -> p h d", d=D)
                if bv_rep is not None:
                    vs = scratch.tile([128, C], F32, tag="v_scr")
                    nc.vector.tensor_scalar(out=vs[:], in0=ps[:],
                                            scalar1=1.0 / WS, scalar2=None,
                                            op0=MUL)
                    nc.vector.tensor_add(vs[:], vs[:], bv_rep[:])
                    nc.vector.tensor_copy(
                        out=vaug[:, ti, :, 0:D],
                        in_=vs[:].rearrange("p (h d) -> p h d", d=D))
                else:
                    nc.vector.tensor_scalar(
                        out=vaug[:, ti, :, 0:D], in0=ps3,
                        scalar1=1.0 / WS, scalar2=None, op0=MUL)

            # ---- stage C: software-pipelined attention ---------------------
            # Per pair c: scores for BOTH heads on 64-row PE tiles, fillers
            # interleaved at group yields, PV lagged two pairs.
            _grps = ((0,), (1,), (2,), (3,), (4, 5), (6, 7))
            _dbs = [(0, 4), (4, 4), (8, 4), (12, 2), (14, 2)]

            def _dbatch(h):
                for b, (s, n) in enumerate(_dbs):
                    if s <= h < s + n:
                        return b, s
                raise AssertionError

            with tc.tile_pool(name="epool", bufs=4) as e_pool, \
                 tc.tile_pool(name="scrp", bufs=2) as scrp, \
                 tc.tile_pool(name="rbp", bufs=2) as rbp:
                den4s = [mid.tile([n, T], BF16, tag=f"den4_{b}", name=f"den4_{b}")
                         for b, (s, n) in enumerate(_dbs)]
                recip_dram = dram.tile([16, T], BF16)
                egrps = {}   # h -> {j: (e_tile, col offset)}

                def _den_batch(b0, h_last):
                    bn = h_last - b0 + 1
                    _b2, _ = _dbatch(b0)
                    with nc.allow_low_precision(
                            reason="softmax denom recip in bf16: denom is "
                                   "O(64-512), 0.4% rel err fits the budget"):
                        nc.vector.reciprocal(
                            out=den4s[_b2][0:bn, :], in_=den4s[_b2][0:bn, :])
                    nc.sync.dma_start(
                        out=recip_dram[b0:b0 + bn, :],
                        in_=den4s[_b2][0:bn, :])
                    # one rb tile + one full-width multiply per head PAIR
                    # (den batches always start at even h)
                    for hh in range(b0, b0 + bn, 2):
                        rb = rbp.tile([128, T], BF16, tag="rb",
                                      name=f"rb_{hh}")
                        for s in range(2):
                            rsrc = bass.AP(tensor=recip_dram.tensor,
                                           offset=(hh + s) * T,
                                           ap=[[0, 64], [1, T]])
                            nc.sync.dma_start(out=rb[s * 64:s * 64 + 64, :],
                                              in_=rsrc)
                        # yT8 = yT * (1/den), fp8 out. All-SBUF operands, so
                        # this runs on the otherwise-idle GPSIMD instead of
                        # jamming the DVE queue at the attention tail.
                        nc.gpsimd.tensor_mul(
                            yT8[:, hh // 2, :], yT[:, hh // 2, :], rb[:])

                def scores(c):
                    """Scores+exp+mask for heads 2c (rows 0:64) and 2c+1
                    (rows 64:128); yields between groups for PE fillers."""
                    for h in (2 * c, 2 * c + 1):
                        egrps[h] = {}
                    for gi, grp in enumerate(_grps):
                        w_g = sum((8 - j) * 128 for j in grp)
                        for h in (2 * c, 2 * c + 1):
                            koff = (h % 2) * 64
                            ps = ps_main.tile([128, w_g], F32, tag="psmain",
                                              name=f"sp_{h}_{gi}")
                            col = 0
                            offs = []
                            for j in grp:
                                rem = (8 - j) * 128
                                lhsT = qk8[koff:koff + 64, 1, c,
                                           j * 128:(j + 1) * 128]
                                off = col
                                src_off = j * 128
                                while off < col + rem:
                                    n = min(col + rem - off, 512 - (off % 512))
                                    nc.tensor.matmul(
                                        ps[:, off:off + n], lhsT,
                                        qk8[koff:koff + 64, 0, c,
                                            src_off:src_off + n],
                                        start=True, stop=True)
                                    off += n
                                    src_off += n
                                offs.append(col)
                                col += rem
                            e = e_pool.tile([128, w_g], BF16, tag=f"e{gi}",
                                            name=f"e_{h}_{gi}")
                            nc.scalar.activation(out=e[:], in_=ps[:],
                                                 func=AF.Exp, scale=0.125)
                            # causal diag masks: SBUF-only, run on GPSIMD to
                            # keep the DVE free for psum evictions
                            for j, off in zip(grp, offs):
                                nc.gpsimd.tensor_mul(
                                    e[:, off:off + 128], e[:, off:off + 128],
                                    mask_sb[:])
                                egrps[h][j] = (e, off)
                        yield gi

                def pv(c):
                    """PV + eviction + den bookkeeping for heads of pair c."""
                    for h in (2 * c, 2 * c + 1):
                        koff = (h % 2) * 64
                        egrp = egrps[h]
                        ps0 = ps_sm.tile([65, 512], F32, tag="ps1b",
                                         name=f"yt0_{h}")
                        ps1 = ps_sm.tile([65, 512], F32, tag="ps1b",
                                         name=f"yt1_{h}")
                        for j in range(NT):
                            lhsT = vaug[:, j, h, :]
                            et, eo = egrp[j]
                            if j <= 3:
                                nA = (4 - j) * 128
                                nc.tensor.matmul(
                                    ps0[:, j * 128:512], lhsT,
                                    et[:, eo:eo + nA],
                                    start=(j == 0), stop=(j == 3))
                                nc.tensor.matmul(
                                    ps1[:, 0:512], lhsT,
                                    et[:, eo + nA:eo + nA + 512],
                                    start=(j == 0), stop=False)
                            else:
                                nB = (8 - j) * 128
                                nc.tensor.matmul(
                                    ps1[:, j * 128 - 512:512], lhsT,
                                    et[:, eo:eo + nB],
                                    start=False, stop=(j == NT - 1))

                        scr = scrp.tile([65, T], BF16, tag="scr", name=f"scr_{h}")
                        nc.vector.tensor_copy(out=scr[:, 0:512], in_=ps0[:])
                        nc.vector.tensor_copy(out=scr[:, 512:1024], in_=ps1[:])
                        _b, _s = _dbatch(h)
                        nc.sync.dma_start(
                            out=den4s[_b][h - _s:h - _s + 1, :],
                            in_=scr[64:65, :])
                        nc.sync.dma_start(
                            out=yT[koff:koff + 64, h // 2, :], in_=scr[0:64, :])

                    # fire any den batch whose pairs have all completed PV
                    pv.done.add(c)
                    for b, (s, n) in enumerate(_dbs):
                        if b in pv.fired:
                            continue
                        prs = {s // 2, (s + n - 1) // 2}
                        if prs <= pv.done:
                            pv.fired.add(b)
                            _den_batch(s, s + n - 1)
                pv.done = set()
                pv.fired = set()

                # Pair processing order: 4,5,6,7 first so their den
                # normalizations (gating the IN-attention proj k4-7 partials)
                # land mid-attention; 0-3 last, their dens gating only the
                # short stage-D k0-3 chains. Fillers keyed by POSITION p:
                #   p0-1: V chunks; p1-5: qkT halves for the late pairs
                #   0-3 (2 positions ahead of their scores); p6-7: proj k4-7
                #   partials. pv(PORD[p-2]) at gi==4.
                PORD = (4, 5, 6, 7, 0, 1, 2, 3)
                _fill = {
                    (0, 0): lambda: v_chunk(0), (0, 1): lambda: v_chunk(1),
                    (0, 2): lambda: v_chunk(2), (0, 3): lambda: v_chunk(3),
                    (0, 4): lambda: v_chunk(4),
                    (1, 0): lambda: v_chunk(5), (1, 1): lambda: v_chunk(6),
                    (1, 2): lambda: v_chunk(7),
                    (1, 3): lambda: qkT_chunk_half(0, 0, 0),
                    (1, 4): lambda: qkT_chunk_half(0, 0, 1),
                    (2, 0): lambda: qkT_chunk_half(1, 0, 0),
                    (2, 1): lambda: qkT_chunk_half(1, 0, 1),
                    (2, 3): lambda: qkT_chunk_half(0, 1, 0),
                    (2, 5): lambda: qkT_chunk_half(0, 1, 1),
                    (3, 0): lambda: qkT_chunk_half(1, 1, 0),
                    (3, 1): lambda: qkT_chunk_half(1, 1, 1),
                    (3, 3): lambda: qkT_chunk_half(0, 2, 0),
                    (3, 5): lambda: qkT_chunk_half(0, 2, 1),
                    (4, 0): lambda: qkT_chunk_half(1, 2, 0),
                    (4, 1): lambda: qkT_chunk_half(1, 2, 1),
                    (4, 3): lambda: qkT_chunk_half(0, 3, 0),
                    (4, 5): lambda: qkT_chunk_half(0, 3, 1),
                    (5, 0): lambda: qkT_chunk_half(1, 3, 0),
                    (5, 1): lambda: qkT_chunk_half(1, 3, 1),
                    (6, 0): lambda: proj_hi(0),
                    (6, 1): lambda: proj_hi(1),
                    (6, 3): lambda: proj_hi(2),
                    (6, 5): lambda: proj_hi(3),
                    (7, 0): lambda: proj_hi(4),
                    (7, 1): lambda: proj_hi(5),
                    (7, 3): lambda: proj_hi(6),
                    (7, 5): lambda: proj_hi(7),
                }
                for p, c in enumerate(PORD):
                    for gi in scores(c):
                        f = _fill.get((p, gi))
                        if f is not None:
                            f()
                        if gi == 4 and p >= 2:
                            pv(PORD[p - 2])
                pv(PORD[NC_ - 2])
                pv(PORD[NC_ - 1])

        # ---- stage D: proj k0-3 -> r1, LN2 -> h2T (bf16) -------------------
        # k0,1 DR chains run 2 deep; k2,3 (gated by the LAST den batch, pairs
        # 2-3) lagged one chunk behind. Plain tensor_add eviction.
        h_blks = {}
        chains = {}

        def _finish(j):
            ps = chains.pop(j)
            lhsT = yT8[:, 2:4, j * 128:(j + 1) * 128]
            for off in (0, 512):
                nc.tensor.matmul(ps[:, off:off + 512], lhsT,
                                 wp_lo[:, 2:4, off:off + 512],
                                 start=False, stop=True, perf_mode=DR)
            nc.vector.tensor_add(x_sb[:, j, :], ps[:], x_sb[:, j, :])
            if bproj_rep is not None:
                nc.vector.tensor_add(x_sb[:, j, :], x_sb[:, j, :],
                                     bproj_rep[:])
            h_blks[j] = layernorm_chunk(x_sb[:, j, :], ln2_g_rep, ln2_b_rep,
                                        BF16)
            if j > 0:
                transpose_into(ps_sm, bufT16, j - 1, h_blks.pop(j - 1),
                               ident, BF16, use_act=True)

        for i in range(NT):
            ps = ps_main.tile([128, C], F32, tag="psmain", name=f"pspj{i}")
            chains[i] = ps
            lhsT01 = yT8[:, 0:2, i * 128:(i + 1) * 128]
            for off in (0, 512):
                nc.tensor.matmul(ps[:, off:off + 512], lhsT01,
                                 wp_lo[:, 0:2, off:off + 512],
                                 start=True, stop=False, perf_mode=DR)
            if i >= 1:
                _finish(i - 1)
        _finish(NT - 1)
        transpose_into(ps_sm, bufT16, NT - 1, h_blks.pop(NT - 1), ident, BF16,
                       use_act=True)
        wps_ctx.close()
        wplo_ctx.close()
        wqks_ctx.close()

        # ---- stage E: fc1 + gelu -> aT (bf16, unchanged) -------------------
        with tc.tile_pool(name="atp", bufs=1) as atp:
            aT = atp.tile([128, 4 * NC_, T], BF16, tag="aT")
            with tc.tile_pool(name="wfcs", bufs=3) as wfcs:
                for m in range(4 * NC_):
                    wf = wfcs.tile([128, NC_, 128], BF16, tag="wf")
                    nc.sync.dma_start(out=wf[:], in_=wfc_d[m])
                    ps = ps_main.tile([128, T], F32, tag="psmain",
                                      name=f"psf1_{m}")
                    for k in range(NC_):
                        for off, n in ((0, 512), (512, 512)):
                            nc.tensor.matmul(ps[:, off:off + n], wf[:, k, :],
                                             bufT16[:, k, off:off + n],
                                             start=(k == 0), stop=(k == NC_ - 1))
                    bias = bfc_sb[:, m:m + 1] if bfc_sb is not None else 0.0
                    nc.scalar.activation(out=aT[:, m, :], in_=ps[:],
                                         func=AF.Gelu_apprx_tanh, bias=bias)

            # ---- stage F: fc2 + residual -> out (2 column passes) ----------
            wm_ctx = ExitStack()
            wmlps = wm_ctx.enter_context(tc.tile_pool(name="wmlps", bufs=3))
            _pre = {}
            for pk in ((0, 0), (0, 1)):
                t = wmlps.tile([128, 512], BF16, tag="wm")
                nc.sync.dma_start(out=t[:], in_=wmlp_d[pk[0], pk[1]])
                _pre[pk] = t
            ps_ctx.close()
            with tc.tile_pool(name="opool", bufs=4) as o_pool, \
                 tc.tile_pool(name="ps_fc2", bufs=8, space="PSUM") as ps_fc2:
                for half in range(2):
                    hoff = half * 512
                    psums = {}
                    for i in range(NT):
                        psums[i] = ps_fc2.tile([128, 512], F32, tag="psf2",
                                               name=f"psf2_{half}_{i}")

                    def _evict(i):
                        o = o_pool.tile([128, 512], F32, tag="o")
                        nc.vector.tensor_add(o[:], psums[i][:],
                                             x_sb[:, i, hoff:hoff + 512])
                        if bmlp_rep is not None:
                            nc.vector.tensor_add(o[:], o[:],
                                                 bmlp_rep[:, hoff:hoff + 512])
                        nc.sync.dma_start(
                            out=out_d[i * 128:(i + 1) * 128, hoff:hoff + 512],
                            in_=o[:])

                    for k in range(4 * NC_):
                        last = k == 4 * NC_ - 1
                        wm = _pre.get((half, k))
                        if wm is None:
                            wm = wmlps.tile([128, 512], BF16, tag="wm")
                            nc.sync.dma_start(out=wm[:], in_=wmlp_d[half, k])
                        for i in range(NT):
                            lhsT = aT[:, k, i * 128:(i + 1) * 128]
                            nc.tensor.matmul(psums[i][:], lhsT, wm[:],
                                             start=(k == 0), stop=last)
                            if last:
                                _evict(i)
            wm_ctx.close()


# ---------------------------------------------------------------------------
_CACHE = {}
_F8NP = ml_dtypes.float8_e4m3


def _q8(a, scale):
    return (np.asarray(a, np.float32) * scale).astype(_F8NP)


def _prearrange_kxm_f8(w, nm, scale):
    """[C, nm*128] -> [nm, 128, C//128, 128] fp8 so chunk DMAs are contiguous.

    out[m, p, ko, mm] = w[ko*128 + p, m*128 + mm] * scale
    """
    cin = w.shape[0]
    a = w.reshape(cin // 128, 128, nm, 128)        # [ko, p, m, mm]
    a = np.transpose(a, (2, 1, 0, 3))              # [m, p, ko, mm]
    return np.ascontiguousarray(_q8(a, scale))


def _prearrange_kxm_bf16(w, nm):
    cin = w.shape[0]
    a = w.reshape(cin // 128, 128, nm, 128)
    a = np.transpose(a, (2, 1, 0, 3))
    return np.ascontiguousarray(a.astype(ml_dtypes.bfloat16))


def _prearrange_mlp(w):
    """[4C, C] -> [2, 4C//128, 128, 512] bf16 column halves (fc2 passes)."""
    a = w.reshape(4 * NC_, 128, 2, 512)            # [k, p, half, n]
    a = np.transpose(a, (2, 0, 1, 3))              # [half, k, p, n]
    return np.ascontiguousarray(a.astype(ml_dtypes.bfloat16))


def _build_in_maps(inputs):
    x = np.asarray(inputs["x"], dtype=np.float32)
    w_qkv = np.asarray(inputs["w_qkv"], dtype=np.float32)
    b_qkv = np.asarray(inputs["b_qkv"], dtype=np.float32)

    flags = {
        "b_qk": bool(np.any(b_qkv[:2 * C])),
        "b_v": bool(np.any(b_qkv[2 * C:])),
        "b_proj": bool(np.any(inputs["b_attn_proj"])),
        "b_fc": bool(np.any(inputs["b_fc"])),
        "b_mlp": bool(np.any(inputs["b_mlp_proj"])),
        "ln1_g": not bool(np.allclose(np.asarray(inputs["ln1_g"]), 1.0)),
        "ln1_b": bool(np.any(inputs["ln1_b"])),
        "ln2_g": not bool(np.allclose(np.asarray(inputs["ln2_g"]), 1.0)),
        "ln2_b": bool(np.any(inputs["ln2_b"])),
    }

    bf = ml_dtypes.bfloat16
    shared = {
        "w_qk": _prearrange_kxm_f8(w_qkv[:, :2 * C], 2 * NC_, WS),
        "w_fc": _prearrange_kxm_bf16(np.asarray(inputs["w_fc"], np.float32),
                                     4 * NC_),
        "w_v": np.ascontiguousarray(_q8(w_qkv[:, 2 * C:], WS)),
        "w_proj": np.ascontiguousarray(
            _q8(np.asarray(inputs["w_attn_proj"], np.float32), 1.0)),
        "w_mlp": _prearrange_mlp(np.asarray(inputs["w_mlp_proj"], np.float32)),
        "mask_ut": np.triu(np.ones((128, 128))).astype(bf),
    }
    if flags["b_qk"]:
        shared["b_qk"] = np.ascontiguousarray(b_qkv[:2 * C].reshape(2 * NC_, 128).T)
    if flags["b_v"]:
        shared["b_v"] = np.ascontiguousarray(b_qkv[2 * C:])
    if flags["b_proj"]:
        shared["b_proj"] = np.asarray(inputs["b_attn_proj"], np.float32)
    if flags["b_fc"]:
        shared["b_fc"] = np.ascontiguousarray(
            np.asarray(inputs["b_fc"], np.float32).reshape(4 * NC_, 128).T)
    if flags["b_mlp"]:
        shared["b_mlp"] = np.asarray(inputs["b_mlp_proj"], np.float32)
    for nm in ("ln1_g", "ln1_b", "ln2_g", "ln2_b"):
        if flags[nm]:
            shared[nm] = np.asarray(inputs[nm], np.float32)

    in_maps = [dict(shared, x=np.ascontiguousarray(x[c])) for c in range(x.shape[0])]
    return flags, in_maps


def kernel_run(inputs, trace=False, trace_kwargs=None):
    """Build (cached), run on 8 cores, return (full_output, BassKernelResults)."""
    from concourse.bass_utils import run_bass_kernel_spmd

    flags, in_maps = _build_in_maps(inputs)
    key = tuple(sorted(flags.items()))
    if key not in _CACHE:
        _CACHE[key] = build_nc(flags)
    nc = _CACHE[key]
    res = run_bass_kernel_spmd(nc, in_maps, core_ids=list(range(8)),
                               trace=trace, trace_kwargs=trace_kwargs or {})
    out = np.stack([res.results[c]["out"] for c in range(8)]).astype(np.float32)
    return out, res


def kernel(**inputs) -> np.ndarray:
    out, _ = kernel_run(inputs, trace=False)
    return out


# revision 58
# speedup vs baseline: 1.0576x; 1.0576x over previous
"""Trainium2 Bass kernel for a dense transformer block (B=8, T=1024, C=1024, H=16).

Data-parallel over batch across the 8 NeuronCores (one batch element per core,
weights replicated, no collectives).

v2: qkT / V / attn-proj matmuls run in fp8e4 DoubleRow (2 k-chunks per PE pass,
~1.7x on those GEMMs); fc1/fc2 stay bf16 (fp8 there blows the 2e-2 rel-err
budget; measured sim rel_l2 0.0117 for this split vs 0.0294 with fp8 MLP).
Scales: weights x32 host-side, yT x16 via the softmax-denominator reciprocal
(vaug ones-column = 1/16), un-scaled on eviction (qkT,V: DVE mult 1/32 replaces
the copy; proj: /512 on eviction).

Per-core dataflow:

  stage W   ~40 warmup matmuls on a zeroed tile (HAM un-throttle), own psum
            pool closed before the main pools so nothing serializes on it.
  stage A   per token chunk: LN1 (DVE stats, ACT normalize -> fp8) -> PE
            transpose (fp8) -> bufT8.  x chunk DMAs front-loaded; wv/wp fp8
            weight DMAs stream underneath.
  stage B   qkT chunks for head-pairs 0-3: fp8 DR, lhsT=wqk 2-chunk groups,
            rhs=bufT8 -> psum -> DVE mult 1/32 -> qk8 (fp8, scale 1).
  stage C   software-pipelined attention, pv lagged TWO pairs behind scores
            so PE never waits on the ACT-bound exp:
              scores: both heads of a pair on 64-row PE tiles from qk8 (fp8
                      non-DR, full rate); exp on ACT; diag masked post-exp.
              PV:     [65,512] psum chains, col 64 = denominator/16.
              filler: V chunks (DR) during pairs 0-1, qkT pairs 4-7 half
                      chains (DR) during pairs 1-5, proj k0-3 partials (DR,
                      eviction ps/512 + add on DVE) during pairs 6-7.
              den:    batched reciprocal (bf16) -> DRAM-broadcast -> yT8 =
                      yT * (16/den) in fp8.
  stage D   proj k4-7: k4,5 DR chains 2 deep; k6, k7 fp8 non-DR lagged behind
            the den tail; evict via ACT(scale 1/512)->f32 + DVE add; LN2 ->
            bf16 -> transpose -> bufT16.
  stage E   fc1 + gelu -> aT bf16 (unchanged from v1).
  stage F   fc2 + residual -> out, 2 column passes, 8x 1-bank psums.
"""
import sys

sys.path.insert(0, "/opt/trn_rl_repo")

import numpy as np
import ml_dtypes

import concourse.bass as bass
import concourse.tile as tile
from concourse import mybir
from concourse.masks import make_identity
from concourse.vector_clock import ScopedClock

F32 = mybir.dt.float32
BF16 = mybir.dt.bfloat16
F8 = mybir.dt.float8e4
AF = mybir.ActivationFunctionType
DR = mybir.MatmulPerfMode.DoubleRow
MUL = mybir.AluOpType.mult

T, C, H, D = 1024, 1024, 16, 64
NT = T // 128   # 8 token chunks
NC_ = C // 128  # 8 feature chunks
K8 = 12         # fc2 contraction chunks done in fp8 DR (of 4C/128=32)
KB = 4 * NC_ - K8   # remaining bf16 fc2 chunks
MS = 64.0       # w_mlp fp8 scale; psum rescaled 1/MS between the groups
EPS = 1e-5
WS = 32.0       # fp8 weight scale for w_qk, w_v (stored x32, free DVE unscale)
# w_proj and yT8 are stored UNSCALED (subnormal-range fp8 quantization is as
# accurate as scaled for this data; sim rel_l2 0.0118 either way) so the proj
# psum is exact and its eviction is a single plain tensor_add.

# ---------------------------------------------------------------------------
# Walrus in this container rejects >1 sem-wait per CTRL instruction; split the
# Tile tail-drain's waits across nop carriers.
_MAX_WAITS = 1


def _patched_drain_and_barrier(self, tick_clock, wait_clock):
    nc = self.nc
    carrier = nc.sync.nop(nofuse=True)
    wait_clock.add_sem_waits(carrier.ins, ScopedClock({None: tick_clock.global_clock}))
    si = carrier.ins.sync_info
    waits = list(si.on_wait) if si and si.on_wait else []
    if len(waits) > _MAX_WAITS:
        si.on_wait = waits[:_MAX_WAITS]
        for k in range(_MAX_WAITS, len(waits), _MAX_WAITS):
            extra = nc.sync.nop(nofuse=True)
            esi = extra.ins.sync_info
            if esi is None:
                extra.ins.sync_info = mybir.SyncInfo(
                    on_wait=waits[k:k + _MAX_WAITS], on_update=[]
                )
            else:
                esi.on_wait = waits[k:k + _MAX_WAITS]
    nc.sync.drain()
    nc.all_engine_barrier()
    popped = nc._tile_sem_poison_stack.pop()
    assert popped is self._sem_poison
    nc.clear_and_free_semaphores(list(self.sems.allocated().values()))
    nc.all_engine_barrier()


tile.TileContext._drain_and_barrier = _patched_drain_and_barrier


def _split_sync_waits(nc, max_waits=1):
    """Walrus here rejects >1 sem-wait per instruction; hoist extras onto
    preceding same-engine nops."""
    ctr = 0
    for f in nc.m.functions:
        for b in f.blocks:
            out = []
            for ins in b.instructions:
                si = ins.sync_info
                ws = list(si.on_wait) if si and si.on_wait else []
                if len(ws) > max_waits:
                    extra, keep = ws[:-max_waits], ws[-max_waits:]
                    for i in range(0, len(extra), max_waits):
                        nop = mybir.InstNoOp(
                            name=f"wsplit-{ctr}", engine=ins.engine,
                            sync_info=mybir.SyncInfo(
                                on_wait=extra[i:i + max_waits], on_update=[]))
                        ctr += 1
                        out.append(nop)
                    si.on_wait = keep
                out.append(ins)
            b.instructions = out


def build_nc(flags, split_waits=True):
    nc = bass.Bass()

    x_d = nc.dram_tensor("x", [T, C], F32, kind="ExternalInput")
    # host-prearranged: [m_chunk, p, ko, 128] so per-chunk DMAs are contiguous
    wqk_d = nc.dram_tensor("w_qk", [2 * NC_, 128, NC_, 128], F8,
                           kind="ExternalInput")
    wfc_d = nc.dram_tensor("w_fc", [4 * NC_, 128, NC_, 128], BF16,
                           kind="ExternalInput")
    wv_d = nc.dram_tensor("w_v", [C, C], F8, kind="ExternalInput")
    wp_d = nc.dram_tensor("w_proj", [C, C], F8, kind="ExternalInput")
    # host-prearranged column halves for the 2-pass fc2: first K8 k-chunks in
    # fp8 (paired for DoubleRow, x MS), the rest bf16
    wmlp8_d = nc.dram_tensor("w_mlp8", [2, K8 // 2, 128, 2, 512], F8,
                             kind="ExternalInput")
    wmlp_d = nc.dram_tensor("w_mlp", [2, KB, 128, 512], BF16,
                            kind="ExternalInput")
    mask_d = nc.dram_tensor("mask_ut", [128, 128], BF16, kind="ExternalInput")
    opt = {}
    if flags["b_qk"]:
        opt["b_qk"] = nc.dram_tensor("b_qk", [128, 2 * NC_], F32, kind="ExternalInput")
    if flags["b_v"]:
        opt["b_v"] = nc.dram_tensor("b_v", [C], F32, kind="ExternalInput")
    if flags["b_proj"]:
        opt["b_proj"] = nc.dram_tensor("b_proj", [C], F32, kind="ExternalInput")
    if flags["b_fc"]:
        opt["b_fc"] = nc.dram_tensor("b_fc", [128, 4 * NC_], F32, kind="ExternalInput")
    if flags["b_mlp"]:
        opt["b_mlp"] = nc.dram_tensor("b_mlp", [C], F32, kind="ExternalInput")
    for nm in ("ln1_g", "ln1_b", "ln2_g", "ln2_b"):
        if flags[nm]:
            opt[nm] = nc.dram_tensor(nm, [C], F32, kind="ExternalInput")
    out_d = nc.dram_tensor("out", [T, C], F32, kind="ExternalOutput")

    with tile.TileContext(nc) as tc:
        _build_body(nc, tc, flags, x_d, wqk_d, wv_d, wp_d, wfc_d, wmlp8_d,
                    wmlp_d, mask_d, opt, out_d)
    if split_waits:
        _split_sync_waits(nc)
    return nc


def _build_body(nc, tc, flags, x_d, wqk_d, wv_d, wp_d, wfc_d, wmlp8_d,
                wmlp_d, mask_d, opt, out_d):
    from contextlib import ExitStack

    ctx = ExitStack()
    with ctx:
        const = ctx.enter_context(tc.tile_pool(name="const", bufs=1))
        big = ctx.enter_context(tc.tile_pool(name="big", bufs=1))
        scratch = ctx.enter_context(tc.tile_pool(name="scratch", bufs=2))
        hb_pool = ctx.enter_context(tc.tile_pool(name="hbp", bufs=4))
        small = ctx.enter_context(tc.tile_pool(name="small", bufs=8))
        dram = ctx.enter_context(tc.tile_pool(name="dram", bufs=1, space="DRAM"))

        # ---- stage W: warmup matmuls on own psum pool (closed right away so
        # nothing rotates behind the warm tile) --------------------------------
        zw = const.tile([128, 128], BF16, tag="zw")
        nc.vector.memset(zw[:], 0.0)
        with tc.tile_pool(name="ps_warm", bufs=1, space="PSUM") as ps_warm:
            warm = ps_warm.tile([128, 128], F32, tag="warm", name="warm")
            for _ in range(56):
                nc.tensor.matmul(warm[:], zw[:], zw[:], start=True, stop=True)

        # ---- constants -----------------------------------------------------
        ident = const.tile([128, 128], BF16, tag="ident")
        make_identity(nc, ident)
        mask_sb = const.tile([128, 128], BF16, tag="mask")
        nc.sync.dma_start(mask_sb[:], mask_d[:])
        eps_t = const.tile([128, 1], F32, tag="eps")
        nc.vector.memset(eps_t[:], EPS)
        # dummy sqrt: loads the ACT sqrt/identity table at t~0 (overlapping
        # the x DMA) instead of stalling the first LN chunk
        dums = const.tile([128, 1], F32, tag="dums")
        nc.scalar.activation(out=dums[:], in_=eps_t[:], func=AF.Sqrt,
                             bias=eps_t[:], scale=1.0)

        def rep128(vec_dram):
            t = const.tile([128, C], F32, tag=f"rep_{vec_dram.tensor.name}")
            src = bass.AP(tensor=vec_dram.tensor, offset=0, ap=[[0, 128], [1, C]])
            nc.gpsimd.dma_start(out=t[:], in_=src)
            return t

        ln1_g_rep = rep128(opt["ln1_g"]) if flags["ln1_g"] else None
        ln1_b_rep = rep128(opt["ln1_b"]) if flags["ln1_b"] else None
        ln2_g_rep = rep128(opt["ln2_g"]) if flags["ln2_g"] else None
        ln2_b_rep = rep128(opt["ln2_b"]) if flags["ln2_b"] else None
        bv_rep = rep128(opt["b_v"]) if flags["b_v"] else None
        bproj_rep = rep128(opt["b_proj"]) if flags["b_proj"] else None
        bmlp_rep = rep128(opt["b_mlp"]) if flags["b_mlp"] else None
        bqk_sb = None
        if flags["b_qk"]:
            bqk_sb = const.tile([128, 2 * NC_], F32, tag="bqk")
            nc.sync.dma_start(bqk_sb[:], opt["b_qk"][:])
        bfc_sb = None
        if flags["b_fc"]:
            bfc_sb = const.tile([128, 4 * NC_], F32, tag="bfc")
            nc.sync.dma_start(bfc_sb[:], opt["b_fc"][:])

        # ---- persistent tiles ---------------------------------------------
        x_sb = big.tile([128, NT, C], F32, tag="x")        # x, then r1 in place
        bufT8 = big.tile([128, NC_, T], F8, tag="bufT8")   # h1T fp8
        bufT16 = big.tile([128, NC_, T], BF16, tag="bufT16")  # h2T bf16
        yT = big.tile([128, NC_, T], BF16, tag="yT")       # raw attention out^T
        yT8 = big.tile([128, NC_, T], F8, tag="yT8")       # normalized, x16
        # all qkT chunks: [p, sub(q=0,k=1), pair, t], fp8 scale 1
        qk8 = big.tile([128, 2, NC_, T], F8, tag="qk8")

        def layernorm_chunk(src_slice, g_rep, b_rep, out_dt):
            stats = small.tile([128, 2, 6], F32, tag="bn_stats")
            xr = src_slice.rearrange("p (s f) -> p s f", f=512)
            for s in range(2):
                nc.vector.bn_stats(out=stats[:, s, :], in_=xr[:, s, :])
            mv = small.tile([128, 2], F32, tag="bn_mv")
            nc.vector.bn_aggr(out=mv[:], in_=stats[:])
            rstd = small.tile([128, 1], F32, tag="rstd")
            nc.scalar.activation(out=rstd[:], in_=mv[:, 1:2], func=AF.Sqrt,
                                 bias=eps_t[:], scale=1.0)
            nc.vector.reciprocal(out=rstd[:], in_=rstd[:])
            # nmr = -mu * rstd; normalize on ACT: h = x*rstd + nmr
            nmr = small.tile([128, 1], F32, tag="nmr")
            nc.vector.tensor_scalar(
                out=nmr[:], in0=mv[:, 0:1], scalar1=rstd[:], scalar2=-1.0,
                op0=MUL, op1=MUL)
            h_blk = hb_pool.tile([128, C], out_dt, tag="hblk")
            nc.scalar.activation(out=h_blk[:], in_=src_slice, func=AF.Identity,
                                 bias=nmr[:], scale=rstd[:])
            if g_rep is not None:
                nc.vector.tensor_mul(h_blk[:], h_blk[:], g_rep[:])
            if b_rep is not None:
                nc.vector.tensor_add(h_blk[:], h_blk[:], b_rep[:])
            return h_blk

        def transpose_into(ps_pool, dst, dst_ti, src_blk, idn, dt, use_act=False):
            for jc in range(NC_):
                pst = ps_pool.tile([128, 128], dt, tag="ps1b")
                nc.tensor.transpose(pst[:], src_blk[:, jc * 128:(jc + 1) * 128],
                                    idn[:])
                d = dst[:, jc, dst_ti * 128:(dst_ti + 1) * 128]
                if use_act and jc % 2 == 1:
                    # ACT has slack in stage D (only the LN2 normalize); DVE
                    # is the busy engine there.
                    nc.scalar.activation(out=d, in_=pst[:], func=AF.Copy)
                else:
                    eng = nc.vector if jc % 2 == 0 else nc.any
                    eng.tensor_copy(out=d, in_=pst[:])

        # Global PSUM pools for stages A-E: three 2-bank "main" slots (the
        # [128, <=1024] f32 accumulators: scores, V, proj chains, fc1) + two
        # 1-bank slots (transposes, PV halves, qkT half-chains, proj pass1)
        # = exactly 8 banks, no stage barriers.
        ps_ctx = ExitStack()
        ps_main = ps_ctx.enter_context(
            tc.tile_pool(name="ps_main", bufs=3, space="PSUM"))
        ps_sm = ps_ctx.enter_context(
            tc.tile_pool(name="ps_sm", bufs=2, space="PSUM"))

        # wp chunk pools open BEFORE wqks so wqks can close right after
        # attention (freeing its slot for the wfc prefetch pool).
        wplo_ctx = ExitStack()
        wplo = wplo_ctx.enter_context(tc.tile_pool(name="wplo", bufs=1))
        wp_lo = wplo.tile([128, 4, C], F8, tag="wplo")
        wps_ctx = ExitStack()
        wps = wps_ctx.enter_context(tc.tile_pool(name="wps", bufs=1))
        wp_hi = wps.tile([128, 4, C], F8, tag="wphi")

        wqks_ctx = ExitStack()
        wqks = wqks_ctx.enter_context(tc.tile_pool(name="wqks", bufs=2))

        def qkT_evict(ps, sub, c, m, off, n):
            if bqk_sb is not None:
                nc.scalar.activation(out=qk8[:, sub, c, off:off + n],
                                     in_=ps[:, off:off + n] if ps.shape[-1] > n
                                     else ps[:], func=AF.Identity,
                                     bias=bqk_sb[:, m:m + 1], scale=1.0 / WS)
            else:
                src = ps[:, off:off + n] if ps.shape[-1] > n else ps[:]
                nc.vector.tensor_scalar(
                    out=qk8[:, sub, c, off:off + n], in0=src,
                    scalar1=1.0 / WS, scalar2=None, op0=MUL)

        def qkT_chunk_full(sub, c):
            """Whole [128, T] chunk via a 2-bank psum, fp8 DoubleRow."""
            m = sub * NC_ + c
            wq = wqks.tile([128, NC_, 128], F8, tag="wq")
            nc.sync.dma_start(out=wq[:], in_=wqk_d[m])
            ps = ps_main.tile([128, T], F32, tag="psmain", name=f"psqk{m}")
            for g in range(NC_ // 2):
                lhsT = wq[:, 2 * g:2 * g + 2, :]
                for off in (0, 512):
                    nc.tensor.matmul(ps[:, off:off + 512], lhsT,
                                     bufT8[:, 2 * g:2 * g + 2, off:off + 512],
                                     start=(g == 0), stop=(g == NC_ // 2 - 1),
                                     perf_mode=DR)
            qkT_evict(ps, sub, c, m, 0, 512)
            qkT_evict(ps, sub, c, m, 512, 512)

        def qkT_chunk_half(sub, c, half):
            """One [128, 512] half-chain via a 1-bank psum (attention filler)."""
            m = sub * NC_ + c
            if half == 0:
                wq = wqks.tile([128, NC_, 128], F8, tag="wq", name=f"wq_{m}")
                nc.sync.dma_start(out=wq[:], in_=wqk_d[m])
                qkT_chunk_half.cur[m] = wq
            wq = qkT_chunk_half.cur[m]
            off = half * 512
            ps = ps_sm.tile([128, 512], F32, tag="ps1b", name=f"psqk{m}_{half}")
            for g in range(NC_ // 2):
                nc.tensor.matmul(ps[:], wq[:, 2 * g:2 * g + 2, :],
                                 bufT8[:, 2 * g:2 * g + 2, off:off + 512],
                                 start=(g == 0), stop=(g == NC_ // 2 - 1),
                                 perf_mode=DR)
            qkT_evict(ps, sub, c, m, off, 512)
        qkT_chunk_half.cur = {}

        def proj_evict_add(i, off, ps):
            nc.vector.tensor_add(x_sb[:, i, off:off + 512], ps[:],
                                 x_sb[:, i, off:off + 512])

        def proj_hi(i):
            """r1[i] partial: += y[:, k4..7] @ Wp rows, fp8 DR, 1-bank psums.
            Runs IN-attention: pairs 4-7 are processed first, so their den
            normalizations land mid-attention."""
            pss = []
            for off in (0, 512):
                pss.append(ps_sm.tile([128, 512], F32, tag="ps1b",
                                      name=f"pjh_{i}_{off}"))
            for g in (0, 1):
                lhsT = yT8[:, 4 + 2 * g:6 + 2 * g, i * 128:(i + 1) * 128]
                for oi, off in enumerate((0, 512)):
                    nc.tensor.matmul(pss[oi][:], lhsT,
                                     wp_hi[:, 2 * g:2 * g + 2, off:off + 512],
                                     start=(g == 0), stop=(g == 1),
                                     perf_mode=DR)
            for oi, off in enumerate((0, 512)):
                proj_evict_add(i, off, pss[oi])

        with tc.tile_pool(name="mid", bufs=1) as mid:
            vaug = mid.tile([128, NT, H, D + 1], BF16, tag="vaug")
            nc.vector.memset(vaug[:, :, :, D:D + 1], 1.0)
            wv_sb = mid.tile([128, NC_, C], F8, tag="wv")

            # ---- stage A: x DMAs, LN1 pipeline (2 ahead), transposes -------
            nc.sync.dma_start(out=x_sb[:, 0, :], in_=x_d[0:128, :])
            for ti in range(1, NT):
                nc.sync.dma_start(out=x_sb[:, ti, :],
                                  in_=x_d[ti * 128:(ti + 1) * 128, :])
            for k in range(NC_):
                nc.sync.dma_start(out=wv_sb[:, k, :],
                                  in_=wv_d[k * 128:(k + 1) * 128, :])
            for k in range(4):
                nc.sync.dma_start(out=wp_lo[:, k, :],
                                  in_=wp_d[k * 128:(k + 1) * 128, :])
                nc.sync.dma_start(out=wp_hi[:, k, :],
                                  in_=wp_d[(k + 4) * 128:(k + 5) * 128, :])
            # transpose runs in bf16 (walrus rejects fp8 transpose-mode psum
            # output); the DVE eviction copy converts bf16 psum -> fp8 bufT8.
            hb1 = {t: layernorm_chunk(x_sb[:, t, :], ln1_g_rep, ln1_b_rep,
                                      BF16) for t in range(3)}
            for ti in range(NT):
                if ti + 3 < NT:
                    hb1[ti + 3] = layernorm_chunk(
                        x_sb[:, ti + 3, :], ln1_g_rep, ln1_b_rep, BF16)
                transpose_into(ps_sm, bufT8, ti, hb1.pop(ti), ident, BF16)

            # dummy exp: pulls the ACT exp-table load into stage B (ACT idle)
            # instead of stalling the first scores eviction by ~2.5us
            dume = small.tile([128, 1], F32, tag="dume")
            nc.scalar.activation(out=dume[:], in_=eps_t[:], func=AF.Exp)

            # ---- stage B: qkT chunks for pairs 4-7 (dense DR; these pairs
            # are processed first in attention) -------------------------------
            for c in (4, 5, 6, 7):
                for sub in range(2):
                    qkT_chunk_full(sub, c)

            def v_chunk(ti):
                """V(ti) fp8 DR: lhsT=h1T chunk, rhs=Wv; evict *1/WS -> vaug."""
                ps = ps_main.tile([128, C], F32, tag="psmain", name=f"psv{ti}")
                for g in range(NC_ // 2):
                    lhsT = bufT8[:, 2 * g:2 * g + 2, ti * 128:(ti + 1) * 128]
                    for off in (0, 512):
                        nc.tensor.matmul(ps[:, off:off + 512], lhsT,
                                         wv_sb[:, 2 * g:2 * g + 2, off:off + 512],
                                         start=(g == 0),
                                         stop=(g == NC_ // 2 - 1),
                                         perf_mode=DR)
                ps3 = ps[:].rearrange("p (h d) -> p h d", d=D)
                if bv_rep is not None:
                    vs = scratch.tile([128, C], F32, tag="v_scr")
                    nc.vector.tensor_scalar(out=vs[:], in0=ps[:],
                                            scalar1=1.0 / WS, scalar2=None,
                                            op0=MUL)
                    nc.vector.tensor_add(vs[:], vs[:], bv_rep[:])
                    nc.vector.tensor_copy(
                        out=vaug[:, ti, :, 0:D],
                        in_=vs[:].rearrange("p (h d) -> p h d", d=D))
                else:
                    nc.vector.tensor_scalar(
                        out=vaug[:, ti, :, 0:D], in0=ps3,
                        scalar1=1.0 / WS, scalar2=None, op0=MUL)

            # ---- stage C: software-pipelined attention ---------------------
            # Per pair c: scores for BOTH heads on 64-row PE tiles, fillers
            # interleaved at group yields, PV lagged two pairs.
            _grps = ((0,), (1,), (2,), (3,), (4, 5), (6, 7))
            _dbs = [(0, 4), (4, 4), (8, 4), (12, 2), (14, 2)]

            def _dbatch(h):
                for b, (s, n) in enumerate(_dbs):
                    if s <= h < s + n:
                        return b, s
                raise AssertionError

            with tc.tile_pool(name="epool", bufs=4) as e_pool, \
                 tc.tile_pool(name="scrp", bufs=2) as scrp, \
                 tc.tile_pool(name="rbp", bufs=2) as rbp:
                den4s = [mid.tile([n, T], BF16, tag=f"den4_{b}", name=f"den4_{b}")
                         for b, (s, n) in enumerate(_dbs)]
                recip_dram = dram.tile([16, T], BF16)
                egrps = {}   # h -> {j: (e_tile, col offset)}

                def _den_batch(b0, h_last):
                    bn = h_last - b0 + 1
                    _b2, _ = _dbatch(b0)
                    with nc.allow_low_precision(
                            reason="softmax denom recip in bf16: denom is "
                                   "O(64-512), 0.4% rel err fits the budget"):
                        nc.vector.reciprocal(
                            out=den4s[_b2][0:bn, :], in_=den4s[_b2][0:bn, :])
                    nc.sync.dma_start(
                        out=recip_dram[b0:b0 + bn, :],
                        in_=den4s[_b2][0:bn, :])
                    # one rb tile + one full-width multiply per head PAIR
                    # (den batches always start at even h)
                    for hh in range(b0, b0 + bn, 2):
                        rb = rbp.tile([128, T], BF16, tag="rb",
                                      name=f"rb_{hh}")
                        for s in range(2):
                            rsrc = bass.AP(tensor=recip_dram.tensor,
                                           offset=(hh + s) * T,
                                           ap=[[0, 64], [1, T]])
                            nc.sync.dma_start(out=rb[s * 64:s * 64 + 64, :],
                                              in_=rsrc)
                        # yT8 = yT * (1/den), fp8 out. All-SBUF operands, so
                        # this runs on the otherwise-idle GPSIMD instead of
                        # jamming the DVE queue at the attention tail.
                        nc.gpsimd.tensor_mul(
                            yT8[:, hh // 2, :], yT[:, hh // 2, :], rb[:])

                def scores(c):
                    """Scores+exp+mask for heads 2c (rows 0:64) and 2c+1
                    (rows 64:128); yields between groups for PE fillers."""
                    for h in (2 * c, 2 * c + 1):
                        egrps[h] = {}
                    for gi, grp in enumerate(_grps):
                        w_g = sum((8 - j) * 128 for j in grp)
                        for h in (2 * c, 2 * c + 1):
                            koff = (h % 2) * 64
                            ps = ps_main.tile([128, w_g], F32, tag="psmain",
                                              name=f"sp_{h}_{gi}")
                            col = 0
                            offs = []
                            for j in grp:
                                rem = (8 - j) * 128
                                lhsT = qk8[koff:koff + 64, 1, c,
                                           j * 128:(j + 1) * 128]
                                off = col
                                src_off = j * 128
                                while off < col + rem:
                                    n = min(col + rem - off, 512 - (off % 512))
                                    nc.tensor.matmul(
                                        ps[:, off:off + n], lhsT,
                                        qk8[koff:koff + 64, 0, c,
                                            src_off:src_off + n],
                                        start=True, stop=True)
                                    off += n
                                    src_off += n
                                offs.append(col)
                                col += rem
                            e = e_pool.tile([128, w_g], BF16, tag=f"e{gi}",
                                            name=f"e_{h}_{gi}")
                            nc.scalar.activation(out=e[:], in_=ps[:],
                                                 func=AF.Exp, scale=0.125)
                            # causal diag masks: SBUF-only, run on GPSIMD to
                            # keep the DVE free for psum evictions
                            for j, off in zip(grp, offs):
                                nc.gpsimd.tensor_mul(
                                    e[:, off:off + 128], e[:, off:off + 128],
                                    mask_sb[:])
                                egrps[h][j] = (e, off)
                        yield gi

                def pv(c):
                    """PV + eviction + den bookkeeping for heads of pair c."""
                    for h in (2 * c, 2 * c + 1):
                        koff = (h % 2) * 64
                        egrp = egrps[h]
                        ps0 = ps_sm.tile([65, 512], F32, tag="ps1b",
                                         name=f"yt0_{h}")
                        ps1 = ps_sm.tile([65, 512], F32, tag="ps1b",
                                         name=f"yt1_{h}")
                        for j in range(NT):
                            lhsT = vaug[:, j, h, :]
                            et, eo = egrp[j]
                            if j <= 3:
                                nA = (4 - j) * 128
                                nc.tensor.matmul(
                                    ps0[:, j * 128:512], lhsT,
                                    et[:, eo:eo + nA],
                                    start=(j == 0), stop=(j == 3))
                                nc.tensor.matmul(
                                    ps1[:, 0:512], lhsT,
                                    et[:, eo + nA:eo + nA + 512],
                                    start=(j == 0), stop=False)
                            else:
                                nB = (8 - j) * 128
                                nc.tensor.matmul(
                                    ps1[:, j * 128 - 512:512], lhsT,
                                    et[:, eo:eo + nB],
                                    start=False, stop=(j == NT - 1))

                        scr = scrp.tile([65, T], BF16, tag="scr", name=f"scr_{h}")
                        nc.vector.tensor_copy(out=scr[:, 0:512], in_=ps0[:])
                        nc.vector.tensor_copy(out=scr[:, 512:1024], in_=ps1[:])
                        _b, _s = _dbatch(h)
                        nc.sync.dma_start(
                            out=den4s[_b][h - _s:h - _s + 1, :],
                            in_=scr[64:65, :])
                        nc.sync.dma_start(
                            out=yT[koff:koff + 64, h // 2, :], in_=scr[0:64, :])

                    # fire any den batch whose pairs have all completed PV
                    pv.done.add(c)
                    for b, (s, n) in enumerate(_dbs):
                        if b in pv.fired:
                            continue
                        prs = {s // 2, (s + n - 1) // 2}
                        if prs <= pv.done:
                            pv.fired.add(b)
                            _den_batch(s, s + n - 1)
                pv.done = set()
                pv.fired = set()

                # Pair processing order: 4,5,6,7 first so their den
                # normalizations (gating the IN-attention proj k4-7 partials)
                # land mid-attention; 0-3 last, their dens gating only the
                # short stage-D k0-3 chains. Fillers keyed by POSITION p:
                #   p0-1: V chunks; p1-5: qkT halves for the late pairs
                #   0-3 (2 positions ahead of their scores); p6-7: proj k4-7
                #   partials. pv(PORD[p-2]) at gi==4.
                PORD = (4, 5, 6, 7, 0, 1, 2, 3)
                _fill = {
                    (0, 0): lambda: v_chunk(0), (0, 1): lambda: v_chunk(1),
                    (0, 2): lambda: v_chunk(2), (0, 3): lambda: v_chunk(3),
                    (0, 4): lambda: v_chunk(4),
                    (1, 0): lambda: v_chunk(5), (1, 1): lambda: v_chunk(6),
                    (1, 2): lambda: v_chunk(7),
                    (1, 3): lambda: qkT_chunk_half(0, 0, 0),
                    (1, 4): lambda: qkT_chunk_half(0, 0, 1),
                    (2, 0): lambda: qkT_chunk_half(1, 0, 0),
                    (2, 1): lambda: qkT_chunk_half(1, 0, 1),
                    (2, 3): lambda: qkT_chunk_half(0, 1, 0),
                    (2, 5): lambda: qkT_chunk_half(0, 1, 1),
                    (3, 0): lambda: qkT_chunk_half(1, 1, 0),
                    (3, 1): lambda: qkT_chunk_half(1, 1, 1),
                    (3, 3): lambda: qkT_chunk_half(0, 2, 0),
                    (3, 5): lambda: qkT_chunk_half(0, 2, 1),
                    (4, 0): lambda: qkT_chunk_half(1, 2, 0),
                    (4, 1): lambda: qkT_chunk_half(1, 2, 1),
                    (4, 3): lambda: qkT_chunk_half(0, 3, 0),
                    (4, 5): lambda: qkT_chunk_half(0, 3, 1),
                    (5, 0): lambda: qkT_chunk_half(1, 3, 0),
                    (5, 1): lambda: qkT_chunk_half(1, 3, 1),
                    (6, 0): lambda: proj_hi(0),
                    (6, 1): lambda: proj_hi(1),
                    (6, 3): lambda: proj_hi(2),
                    (6, 5): lambda: proj_hi(3),
                    (7, 0): lambda: proj_hi(4),
                    (7, 1): lambda: proj_hi(5),
                    (7, 3): lambda: proj_hi(6),
                    (7, 5): lambda: proj_hi(7),
                }
                for p, c in enumerate(PORD):
                    for gi in scores(c):
                        f = _fill.get((p, gi))
                        if f is not None:
                            f()
                        if gi == 4 and p >= 2:
                            pv(PORD[p - 2])
                pv(PORD[NC_ - 2])
                pv(PORD[NC_ - 1])

        # attention pools closed; free wqks and prefetch the first fc1
        # weight chunks so stage E starts without a DMA bubble
        wqks_ctx.close()
        wfcs_ctx = ExitStack()
        wfcs = wfcs_ctx.enter_context(tc.tile_pool(name="wfcs", bufs=3))
        _wf_pre = {}
        for m in (0, 1):
            wf = wfcs.tile([128, NC_, 128], BF16, tag="wf")
            nc.sync.dma_start(out=wf[:], in_=wfc_d[m])
            _wf_pre[m] = wf

        # ---- stage D: proj k0-3 -> r1, LN2 -> h2T (bf16) -------------------
        # k0,1 DR chains run 2 deep; k2,3 (gated by the LAST den batch, pairs
        # 2-3) lagged one chunk behind. Plain tensor_add eviction.
        h_blks = {}
        chains = {}

        def _finish(j):
            ps = chains.pop(j)
            lhsT = yT8[:, 2:4, j * 128:(j + 1) * 128]
            for off in (0, 512):
                nc.tensor.matmul(ps[:, off:off + 512], lhsT,
                                 wp_lo[:, 2:4, off:off + 512],
                                 start=False, stop=True, perf_mode=DR)
            nc.vector.tensor_add(x_sb[:, j, :], ps[:], x_sb[:, j, :])
            if bproj_rep is not None:
                nc.vector.tensor_add(x_sb[:, j, :], x_sb[:, j, :],
                                     bproj_rep[:])
            h_blks[j] = layernorm_chunk(x_sb[:, j, :], ln2_g_rep, ln2_b_rep,
                                        BF16)
            if j > 0:
                transpose_into(ps_sm, bufT16, j - 1, h_blks.pop(j - 1),
                               ident, BF16, use_act=True)

        for i in range(NT):
            ps = ps_main.tile([128, C], F32, tag="psmain", name=f"pspj{i}")
            chains[i] = ps
            lhsT01 = yT8[:, 0:2, i * 128:(i + 1) * 128]
            for off in (0, 512):
                nc.tensor.matmul(ps[:, off:off + 512], lhsT01,
                                 wp_lo[:, 0:2, off:off + 512],
                                 start=True, stop=False, perf_mode=DR)
            if i >= 1:
                _finish(i - 1)
        _finish(NT - 1)
        transpose_into(ps_sm, bufT16, NT - 1, h_blks.pop(NT - 1), ident, BF16,
                       use_act=True)

        # ---- stage E: fc1 + gelu -> aT8 (first K8 chunks, fp8) / aT (bf16) -
        with tc.tile_pool(name="atp", bufs=1) as atp:
            aT8 = atp.tile([128, K8, T], F8, tag="aT8")
            aT = atp.tile([128, KB, T], BF16, tag="aT")
            for m in range(4 * NC_):
                wf = _wf_pre.pop(m, None)
                if wf is None:
                    wf = wfcs.tile([128, NC_, 128], BF16, tag="wf")
                    nc.sync.dma_start(out=wf[:], in_=wfc_d[m])
                ps = ps_main.tile([128, T], F32, tag="psmain",
                                  name=f"psf1_{m}")
                for k in range(NC_):
                    for off, n in ((0, 512), (512, 512)):
                        nc.tensor.matmul(ps[:, off:off + n], wf[:, k, :],
                                         bufT16[:, k, off:off + n],
                                         start=(k == 0), stop=(k == NC_ - 1))
                bias = bfc_sb[:, m:m + 1] if bfc_sb is not None else 0.0
                dst = aT8[:, m, :] if m < K8 else aT[:, m - K8, :]
                nc.scalar.activation(out=dst, in_=ps[:],
                                     func=AF.Gelu_apprx_tanh, bias=bias)

            # ---- stage F: fc2 + residual -> out (2 column passes) ----------
            # First K8 contraction chunks run fp8 DR with Wmlp x MS; the psum
            # is then rescaled 1/MS in place on the (idle) DVE before the
            # bf16 chunks accumulate on top.
            wm_ctx = ExitStack()
            wmlps = wm_ctx.enter_context(tc.tile_pool(name="wmlps", bufs=3))
            _pre8 = {}
            for pk in ((0, 0), (0, 1)):
                t = wmlps.tile([128, 2, 512], F8, tag="wm8")
                nc.sync.dma_start(out=t[:], in_=wmlp8_d[pk[0], pk[1]])
                _pre8[pk] = t
            ps_ctx.close()
            with tc.tile_pool(name="opool", bufs=4) as o_pool, \
                 tc.tile_pool(name="ps_fc2", bufs=8, space="PSUM") as ps_fc2:
                for half in range(2):
                    hoff = half * 512
                    psums = {}
                    for i in range(NT):
                        psums[i] = ps_fc2.tile([128, 512], F32, tag="psf2",
                                               name=f"psf2_{half}_{i}")

                    def _evict(i):
                        o = o_pool.tile([128, 512], F32, tag="o")
                        nc.vector.tensor_add(o[:], psums[i][:],
                                             x_sb[:, i, hoff:hoff + 512])
                        if bmlp_rep is not None:
                            nc.vector.tensor_add(o[:], o[:],
                                                 bmlp_rep[:, hoff:hoff + 512])
                        nc.sync.dma_start(
                            out=out_d[i * 128:(i + 1) * 128, hoff:hoff + 512],
                            in_=o[:])

                    for g in range(K8 // 2):
                        wm8 = _pre8.get((half, g))
                        if wm8 is None:
                            wm8 = wmlps.tile([128, 2, 512], F8, tag="wm8")
                            nc.sync.dma_start(out=wm8[:], in_=wmlp8_d[half, g])
                        for i in range(NT):
                            lhsT = aT8[:, 2 * g:2 * g + 2,
                                       i * 128:(i + 1) * 128]
                            # stop on the last fp8 group: the DVE rescale
                            # reads the psum; the bf16 chunks then continue
                            # accumulating (start=False, group check off)
                            nc.tensor.matmul(psums[i][:], lhsT, wm8[:],
                                             start=(g == 0),
                                             stop=(g == K8 // 2 - 1),
                                             perf_mode=DR)
                    for i in range(NT):
                        nc.vector.tensor_scalar(
                            out=psums[i][:], in0=psums[i][:],
                            scalar1=1.0 / MS, scalar2=None, op0=MUL)
                    for k in range(KB):
                        last = k == KB - 1
                        wm = wmlps.tile([128, 512], BF16, tag="wm")
                        nc.sync.dma_start(out=wm[:], in_=wmlp_d[half, k])
                        for i in range(NT):
                            lhsT = aT[:, k, i * 128:(i + 1) * 128]
                            nc.tensor.matmul(psums[i][:], lhsT, wm[:],
                                             start=False, stop=last,
                                             skip_group_check=True)
                            if last:
                                _evict(i)
            wm_ctx.close()
        wfcs_ctx.close()
        wps_ctx.close()
        wplo_ctx.close()


# ---------------------------------------------------------------------------
_CACHE = {}
_F8NP = ml_dtypes.float8_e4m3


def _q8(a, scale):
    return (np.asarray(a, np.float32) * scale).astype(_F8NP)


def _prearrange_kxm_f8(w, nm, scale):
    """[C, nm*128] -> [nm, 128, C//128, 128] fp8 so chunk DMAs are contiguous.

    out[m, p, ko, mm] = w[ko*128 + p, m*128 + mm] * scale
    """
    cin = w.shape[0]
    a = w.reshape(cin // 128, 128, nm, 128)        # [ko, p, m, mm]
    a = np.transpose(a, (2, 1, 0, 3))              # [m, p, ko, mm]
    return np.ascontiguousarray(_q8(a, scale))


def _prearrange_kxm_bf16(w, nm):
    cin = w.shape[0]
    a = w.reshape(cin // 128, 128, nm, 128)
    a = np.transpose(a, (2, 1, 0, 3))
    return np.ascontiguousarray(a.astype(ml_dtypes.bfloat16))


def _prearrange_mlp(w):
    """[4C, C] -> fp8 part [2, K8//2, 128, 2, 512] (x MS, DR-paired) + bf16
    part [2, KB, 128, 512]; both split into fc2's two column halves."""
    a8 = w[:K8 * 128, :].reshape(K8 // 2, 2, 128, 2, 512)  # [g, s, p, half, n]
    a8 = np.transpose(a8, (3, 0, 2, 1, 4))                 # [half, g, p, s, n]
    a = w[K8 * 128:, :].reshape(KB, 128, 2, 512)           # [k, p, half, n]
    a = np.transpose(a, (2, 0, 1, 3))                      # [half, k, p, n]
    return (np.ascontiguousarray(_q8(a8, MS)),
            np.ascontiguousarray(a.astype(ml_dtypes.bfloat16)))


def _build_in_maps(inputs):
    x = np.asarray(inputs["x"], dtype=np.float32)
    w_qkv = np.asarray(inputs["w_qkv"], dtype=np.float32)
    b_qkv = np.asarray(inputs["b_qkv"], dtype=np.float32)

    flags = {
        "b_qk": bool(np.any(b_qkv[:2 * C])),
        "b_v": bool(np.any(b_qkv[2 * C:])),
        "b_proj": bool(np.any(inputs["b_attn_proj"])),
        "b_fc": bool(np.any(inputs["b_fc"])),
        "b_mlp": bool(np.any(inputs["b_mlp_proj"])),
        "ln1_g": not bool(np.allclose(np.asarray(inputs["ln1_g"]), 1.0)),
        "ln1_b": bool(np.any(inputs["ln1_b"])),
        "ln2_g": not bool(np.allclose(np.asarray(inputs["ln2_g"]), 1.0)),
        "ln2_b": bool(np.any(inputs["ln2_b"])),
    }

    bf = ml_dtypes.bfloat16
    shared = {
        "w_qk": _prearrange_kxm_f8(w_qkv[:, :2 * C], 2 * NC_, WS),
        "w_fc": _prearrange_kxm_bf16(np.asarray(inputs["w_fc"], np.float32),
                                     4 * NC_),
        "w_v": np.ascontiguousarray(_q8(w_qkv[:, 2 * C:], WS)),
        "w_proj": np.ascontiguousarray(
            _q8(np.asarray(inputs["w_attn_proj"], np.float32), 1.0)),
        "mask_ut": np.triu(np.ones((128, 128))).astype(bf),
    }
    shared["w_mlp8"], shared["w_mlp"] = _prearrange_mlp(
        np.asarray(inputs["w_mlp_proj"], np.float32))
    if flags["b_qk"]:
        shared["b_qk"] = np.ascontiguousarray(b_qkv[:2 * C].reshape(2 * NC_, 128).T)
    if flags["b_v"]:
        shared["b_v"] = np.ascontiguousarray(b_qkv[2 * C:])
    if flags["b_proj"]:
        shared["b_proj"] = np.asarray(inputs["b_attn_proj"], np.float32)
    if flags["b_fc"]:
        shared["b_fc"] = np.ascontiguousarray(
            np.asarray(inputs["b_fc"], np.float32).reshape(4 * NC_, 128).T)
    if flags["b_mlp"]:
        shared["b_mlp"] = np.asarray(inputs["b_mlp_proj"], np.float32)
    for nm in ("ln1_g", "ln1_b", "ln2_g", "ln2_b"):
        if flags[nm]:
            shared[nm] = np.asarray(inputs[nm], np.float32)

    in_maps = [dict(shared, x=np.ascontiguousarray(x[c])) for c in range(x.shape[0])]
    return flags, in_maps


def kernel_run(inputs, trace=False, trace_kwargs=None):
    """Build (cached), run on 8 cores, return (full_output, BassKernelResults)."""
    from concourse.bass_utils import run_bass_kernel_spmd

    flags, in_maps = _build_in_maps(inputs)
    key = tuple(sorted(flags.items()))
    if key not in _CACHE:
        _CACHE[key] = build_nc(flags)
    nc = _CACHE[key]
    res = run_bass_kernel_spmd(nc, in_maps, core_ids=list(range(8)),
                               trace=trace, trace_kwargs=trace_kwargs or {})
    out = np.stack([res.results[c]["out"] for c in range(8)]).astype(np.float32)
    return out, res


def kernel(**inputs) -> np.ndarray:
    out, _ = kernel_run(inputs, trace=False)
    return out
